# revision 1
# baseline (speedup 1.0000x reference)
"""Contrastive-loss Trainium2 kernel: 8-way data-parallel over similarity rows.

Strategy (per sharding hint): each of the 8 NeuronCores computes a
[1024, 8192] block of the similarity matrix sim = e @ e.T / T against the
full embedding matrix, reduces per-row numerator / denominator / validity
on-device, and returns per-partition partial (loss_sum, valid_count); the
host sums the 8x[128,2] partials.

Key layout trick: rows are sorted by label on the host and each core's
input is rolled so its 1024 rows sit at a fixed offset (PAD). Same-label
columns of any 128-row tile then live in a fixed 640-wide window
[t*128, t*128+640), so the label-mask / positive-gate / numerator work
touches 640 instead of 8192 columns per row. The denominator row-sum comes
free from the Exp activation's accum_out. Matmuls run in bf16 (fp32 PSUM
accumulate); everything downstream of exp is fp32.
"""

import contextlib
import ctypes
import os
import sys
import types

import ml_dtypes
import numpy as np

import concourse.bass as bass
import concourse.mybir as mybir
import concourse.tile as tile
from concourse.bass_utils import run_bass_kernel_spmd

# problem constants (hardcoded per task contract)
N, D, NCLS = 8192, 512, 512
TEMP = 0.07
EPS = 1e-8
M = 8            # cores
R = N // M       # 1024 rows per core
NT = R // 128    # 8 row-tiles per core
PAD = 256        # roll margin; must exceed max class size
WIN = 128 + 2 * PAD   # 640 col window containing all same-label cols of a tile
CH = 512         # matmul moving-dim chunk (one PSUM bank)
GRP = 2048       # columns per psum group / exp call (4 banks)
NG = N // GRP    # 4 groups
KT = D // 128    # 4 contraction tiles

_AXON_SO = "/opt/axon/libaxon_pjrt.so"

LAST_RESULTS = None   # BassKernelResults of the most recent run (for test.py)


def _install_axon_trace_hook():
    """Provide antenv.axon_hooks (NTFF profiling) if the image lacks it."""
    try:
        from antenv.axon_hooks import get_axon_ntff_profile_hook  # noqa: F401
        return
    except ImportError:
        pass
    if not os.path.exists(_AXON_SO):
        return
    try:
        lib = ctypes.CDLL(_AXON_SO)
    except OSError:
        return
    if not hasattr(lib, "axon_start_nrt_profile"):
        return
    lib.axon_start_nrt_profile.argtypes = [ctypes.POINTER(ctypes.c_int64), ctypes.c_size_t]
    lib.axon_start_nrt_profile.restype = ctypes.c_int64
    lib.axon_stop_nrt_profile.argtypes = [ctypes.c_char_p]
    lib.axon_stop_nrt_profile.restype = ctypes.c_int64

    @contextlib.contextmanager
    def _hook(output_dir, device_ids):
        import jax
        jax.devices()
        if device_ids:
            ids = (ctypes.c_int64 * len(device_ids))(*device_ids)
            rc = lib.axon_start_nrt_profile(ids, len(device_ids))
        else:
            rc = lib.axon_start_nrt_profile(None, 0)
        if rc != 0:
            raise RuntimeError(f"axon_start_nrt_profile rc={rc}")
        try:
            yield
        finally:
            n = lib.axon_stop_nrt_profile(str(output_dir).encode())
            if n < 0:
                raise RuntimeError(f"axon_stop_nrt_profile rc={n}")

    _the_hook = [_hook]
    mod = types.ModuleType("antenv.axon_hooks")
    mod.set_axon_ntff_profile_hook = lambda h: _the_hook.__setitem__(0, h)
    mod.get_axon_ntff_profile_hook = lambda: _the_hook[0]
    sys.modules["antenv.axon_hooks"] = mod
    import antenv
    antenv.axon_hooks = mod


def _split_excess_waits(nc, max_waits=1):
    """This walrus build allows one sync-wait per instruction; move extras
    onto same-engine NoOps inserted just before (execution order preserved)."""
    for f in nc.m.functions:
        for b in f.blocks:
            insts = b.instructions
            new = []
            changed = False
            for inst in insts:
                si = inst.sync_info
                ow = list(si.on_wait) if (si and si.on_wait) else []
                if len(ow) > max_waits:
                    extra, keep = ow[:-max_waits], ow[-max_waits:]
                    for k, w in enumerate(extra):
                        nop = mybir.InstNoOp(name=f"{inst.name}-w{k}", ins=[], outs=[])
                        nop.engine = inst.engine
                        nop.sync_info = mybir.SyncInfo(on_wait=[w], on_update=[])
                        new.append(nop)
                    inst.sync_info = mybir.SyncInfo(
                        on_wait=keep,
                        on_update=list(si.on_update) if si.on_update else [])
                    changed = True
                new.append(inst)
            if changed:
                b.instructions = new


def _build_nc():
    f32 = mybir.dt.float32
    bf16 = mybir.dt.bfloat16
    Alu = mybir.AluOpType
    Act = mybir.ActivationFunctionType

    nc = bass.Bass(trn_type="TRN2", target_bir_lowering=False, debug=False)
    qT = nc.dram_tensor("qT", [128, (N // CH) * KT * CH], bf16, kind="ExternalInput")
    labd = nc.dram_tensor("lab", [N, 1], f32, kind="ExternalInput")
    antid = nc.dram_tensor("anti", [128, 128], f32, kind="ExternalInput")
    identd = nc.dram_tensor("ident", [128, 128], f32, kind="ExternalInput")
    outd = nc.dram_tensor("out", [128, 2], f32, kind="ExternalOutput")

    with tile.TileContext(nc) as tc, contextlib.ExitStack() as ctx:
        qp = ctx.enter_context(tc.tile_pool(name="qp", bufs=1))
        pp = ctx.enter_context(tc.tile_pool(name="pp", bufs=2, space="PSUM"))
        ep = ctx.enter_context(tc.tile_pool(name="ep", bufs=3))
        wp = ctx.enter_context(tc.tile_pool(name="wp", bufs=2))
        sp = ctx.enter_context(tc.tile_pool(name="sp", bufs=1))

        # ---- preload ----
        # qT chunks: [128, KT, CH] bf16, one per 512-col chunk
        qt = []
        for n in range(N // CH):
            q = qp.tile([128, KT, CH], bf16, tag=f"q{n}")
            nc.sync.dma_start(
                out=q, in_=qT[:, n * KT * CH:(n + 1) * KT * CH])
            qt.append(q)
        # row labels per (partition, tile): lab[PAD + t*128 + p]
        lab_rows = sp.tile([128, NT, 1], f32)
        nc.sync.dma_start(
            out=lab_rows,
            in_=labd[PAD:PAD + R, :].rearrange("(t p) o -> p t o", p=128))
        # column labels broadcast to all partitions, cols [0, NT*128+WIN)
        labw_w = (NT - 1) * 128 + WIN        # 1536
        labw = sp.tile([128, labw_w], f32)
        nc.sync.dma_start(
            out=labw,
            in_=bass.AP(tensor=labd, offset=0, ap=[[0, 128], [1, labw_w]]))
        anti = sp.tile([128, 128], f32)
        nc.sync.dma_start(out=anti, in_=antid.ap())
        ident = sp.tile([128, 128], f32)
        nc.sync.dma_start(out=ident, in_=identd.ap())
        eps_t = sp.tile([128, 1], f32)
        nc.vector.memset(eps_t, EPS)
        warm = sp.tile([128, 128], bf16)
        nc.vector.memset(warm, 0.0)
        warm_ps = pp.tile([128, GRP], f32, tag="ps")
        for w in range(48):
            nc.tensor.matmul(warm_ps[:, :128], warm, warm, start=True, stop=True)

        # ---- accumulators ----
        dacc = sp.tile([128, NT * NG], f32)   # exp row-sums per (t, g)
        nacc = sp.tile([128, NT], f32)        # numerator per t
        edacc = sp.tile([128, NT], f32)       # diagonal exp per t

        # ---- main loop ----
        for t in range(NT):
            a = (PAD + t * 128) // CH        # lhsT chunk index
            off = (PAD + t * 128) % CH       # lhsT offset within chunk
            for g in range(NG):
                ps = pp.tile([128, GRP], f32, tag="ps")
                for sub in range(GRP // CH):
                    n = g * (GRP // CH) + sub
                    for k in range(KT):
                        nc.tensor.matmul(
                            ps[:, sub * CH:(sub + 1) * CH],
                            qt[a][:, k, off:off + 128],
                            qt[n][:, k, :],
                            start=(k == 0), stop=(k == KT - 1))
                e = ep.tile([128, GRP], f32, tag="e")
                nc.scalar.activation(
                    out=e, in_=ps[:], func=Act.Exp, scale=float(1.0 / TEMP),
                    accum_out=dacc[:, t * NG + g:t * NG + g + 1])
                if g == 0:
                    # window = cols [t*128, t*128+WIN) -- inside group 0
                    w0 = t * 128
                    u = wp.tile([128, WIN], f32, tag="u")
                    # u = (lab_col == lab_row) * exp(sim)
                    nc.vector.scalar_tensor_tensor(
                        out=u, in0=labw[:, w0:w0 + WIN],
                        scalar=lab_rows[:, t, :], in1=e[:, w0:w0 + WIN],
                        op0=Alu.is_equal, op1=Alu.mult)
                    # diagonal sits at window cols [PAD, PAD+128)
                    scr = wp.tile([128, 128], f32, tag="scr")
                    nc.vector.scalar_tensor_tensor(
                        out=scr, in0=u[:, PAD:PAD + 128], scalar=1.0,
                        in1=ident, op0=Alu.mult, op1=Alu.mult,
                        accum_out=edacc[:, t:t + 1])
                    nc.vector.tensor_tensor(
                        out=u[:, PAD:PAD + 128], in0=u[:, PAD:PAD + 128],
                        in1=anti, op=Alu.mult)
                    # numerator: sum over u where u > 1  (sim>0 gate)
                    scr2 = wp.tile([128, WIN], f32, tag="scr2")
                    nc.vector.scalar_tensor_tensor(
                        out=scr2, in0=u, scalar=1.0, in1=u,
                        op0=Alu.is_gt, op1=Alu.mult,
                        accum_out=nacc[:, t:t + 1])

        # ---- epilogue (all [128, NT]) ----
        dred = sp.tile([128, NT], f32)
        nc.vector.tensor_reduce(
            out=dred, in_=dacc.rearrange("p (t g) -> p t g", g=NG),
            axis=mybir.AxisListType.X, op=Alu.add)
        den = sp.tile([128, NT], f32)
        nc.vector.tensor_tensor(out=den, in0=dred, in1=edacc, op=Alu.subtract)
        v1 = sp.tile([128, NT], f32)
        nc.vector.tensor_scalar(out=v1, in0=nacc, scalar1=0.0, scalar2=None,
                                op0=Alu.is_gt)
        v2 = sp.tile([128, NT], f32)
        nc.vector.tensor_scalar(out=v2, in0=den, scalar1=0.0, scalar2=None,
                                op0=Alu.is_gt)
        v = sp.tile([128, NT], f32)
        nc.vector.tensor_tensor(out=v, in0=v1, in1=v2, op=Alu.mult)
        inv = sp.tile([128, NT], f32)
        nc.vector.tensor_scalar(out=inv, in0=v, scalar1=0.0, scalar2=None,
                                op0=Alu.is_equal)
        nsafe = sp.tile([128, NT], f32)
        nc.vector.tensor_tensor(out=nsafe, in0=nacc, in1=v, op=Alu.mult)
        nc.vector.tensor_tensor(out=nsafe, in0=nsafe, in1=inv, op=Alu.add)
        dsafe = sp.tile([128, NT], f32)
        nc.vector.tensor_tensor(out=dsafe, in0=den, in1=v, op=Alu.mult)
        nc.vector.tensor_tensor(out=dsafe, in0=dsafe, in1=inv, op=Alu.add)
        lgd = sp.tile([128, NT], f32)
        nc.scalar.activation(out=lgd, in_=dsafe, func=Act.Ln, bias=eps_t[:], scale=1.0)
        lgn = sp.tile([128, NT], f32)
        nc.scalar.activation(out=lgn, in_=nsafe, func=Act.Ln, scale=1.0)
        li = sp.tile([128, NT], f32)
        nc.vector.tensor_tensor(out=li, in0=lgd, in1=lgn, op=Alu.subtract)
        nc.vector.tensor_tensor(out=li, in0=li, in1=v, op=Alu.mult)
        o = sp.tile([128, 2], f32)
        nc.vector.tensor_reduce(out=o[:, 0:1], in_=li, axis=mybir.AxisListType.X,
                                op=Alu.add)
        nc.vector.tensor_reduce(out=o[:, 1:2], in_=v, axis=mybir.AxisListType.X,
                                op=Alu.add)
        nc.sync.dma_start(out=outd.ap(), in_=o)

    _split_excess_waits(nc)
    return nc


_NC = None


def _get_nc():
    global _NC
    if _NC is None:
        _NC = _build_nc()
    return _NC


def _host_reference(emb, lab):
    """Numpy fallback (only for pathological label distributions where a
    class exceeds the PAD margin; never triggers for the target regime)."""
    e = emb / np.linalg.norm(emb, axis=1, keepdims=True).astype(np.float32)
    sim = (e @ e.T).astype(np.float32) / np.float32(TEMP)
    E = np.exp(sim, dtype=np.float32)
    pos = (lab[:, None] == lab[None, :]) & ~np.eye(len(lab), dtype=bool)
    valid = pos & (sim > 0)
    num = np.where(valid, E, 0).sum(1, dtype=np.float32)
    den = E.sum(1, dtype=np.float32) - np.diagonal(E)
    rv = valid.any(1) & (den > 0)
    ns = np.where(rv, num, np.float32(1.0))
    ds = np.where(rv, den, np.float32(1.0))
    li = np.log(ds + np.float32(EPS)) - np.log(ns)
    nv = int(rv.sum())
    if nv == 0:
        return np.float32(0.0)
    return np.float32(abs(float(np.where(rv, li, 0).sum(dtype=np.float64)) / nv))


def kernel(**inputs):
    global LAST_RESULTS
    emb = np.ascontiguousarray(np.asarray(inputs["embeddings"], dtype=np.float32))
    lab = np.asarray(inputs["labels"]).astype(np.int64).ravel()
    assert emb.shape == (N, D) and lab.shape == (N,)

    if np.bincount(lab, minlength=1).max() > PAD:
        return _host_reference(emb, lab)

    _install_axon_trace_hook()

    # host prep: normalize, sort by label, per-core roll + transpose
    e = emb / np.linalg.norm(emb, axis=1, keepdims=True).astype(np.float32)
    order = np.argsort(lab, kind="stable")
    es = np.ascontiguousarray(e[order])
    ls = lab[order].astype(np.float32)

    anti = (1.0 - np.eye(128, dtype=np.float32)).astype(np.float32)
    ident = np.eye(128, dtype=np.float32)

    in_maps = []
    for c in range(M):
        shift = c * R - PAD
        rolled = np.roll(es, -shift, axis=0)         # [N, D] f32
        labr = np.roll(ls, -shift).reshape(N, 1)     # [N, 1] f32
        # [D, N] -> [128, NCH, KT, CH]: partition p, chunk n holds
        # qT[k*128+p, n*CH:(n+1)*CH] contiguckus per (k)
        qTc = (rolled.T.reshape(KT, 128, N // CH, CH)
               .transpose(1, 2, 0, 3)
               .reshape(128, (N // CH) * KT * CH)
               .astype(ml_dtypes.bfloat16))
        qTc = np.ascontiguousarray(qTc)
        in_maps.append({
            "qT": qTc,
            "lab": np.ascontiguousarray(labr),
            "anti": anti,
            "ident": ident,
        })

    nc = _get_nc()
    res = run_bass_kernel_spmd(nc, in_maps, core_ids=list(range(M)))
    LAST_RESULTS = res

    loss_sum = 0.0
    cnt = 0.0
    for c in range(M):
        o = res.results[c]["out"]
        loss_sum += float(o[:, 0].sum(dtype=np.float64))
        cnt += float(o[:, 1].sum(dtype=np.float64))
    if cnt <= 0:
        return np.float32(0.0)
    return np.float32(abs(loss_sum / cnt))



# revision 7
# speedup vs baseline: 1.4161x; 1.4161x over previous
"""Contrastive-loss Trainium2 kernel: 8-way data-parallel over similarity rows.

Strategy (per sharding hint): each of the 8 NeuronCores computes a
[1024, 8192] block of the similarity matrix sim = e @ e.T / T against the
full embedding matrix, reduces per-row numerator / denominator / validity
on-device, and returns per-partition partial (loss_sum, valid_count); the
host sums the 8x[128,2] partials.

Key layout trick: rows are sorted by label on the host and each core's
input is rolled so its 1024 rows sit at a fixed offset (PAD). Same-label
columns of any 128-row tile then live in a fixed 640-wide window
[t*128, t*128+640), so the label-mask / positive-gate / numerator work
touches 640 instead of 8192 columns per row. The denominator row-sum comes
free from the Exp activation's accum_out. Matmuls run in fp8 e4m3 with
DoubleRow perf mode (2 k-subtiles per instruction, fp32 PSUM accumulate,
power-of-two quantization scale folded into the Exp activation scale);
everything downstream of exp is fp32.
"""

import contextlib
import ctypes
import os
import sys
import types

import ml_dtypes
import numpy as np

import concourse.bass as bass
import concourse.mybir as mybir
import concourse.tile as tile
from concourse.bass_utils import run_bass_kernel_spmd

# problem constants (hardcoded per task contract)
N, D, NCLS = 8192, 512, 512
TEMP = 0.07
EPS = 1e-8
M = 8            # cores
R = N // M       # 1024 rows per core
NT = R // 128    # 8 row-tiles per core
PAD = 256        # roll margin; must exceed max class size
WIN = 128 + 2 * PAD   # 640 col window containing all same-label cols of a tile
CH = 512         # matmul moving-dim chunk (one PSUM bank)
GRP = 2048       # columns per psum group / exp call (4 banks)
NG = N // GRP    # 4 groups
KT = D // 128    # 4 contraction tiles

_AXON_SO = "/opt/axon/libaxon_pjrt.so"

LAST_RESULTS = None   # BassKernelResults of the most recent run (for test.py)


def _install_axon_trace_hook():
    """Provide antenv.axon_hooks (NTFF profiling) if the image lacks it."""
    try:
        from antenv.axon_hooks import get_axon_ntff_profile_hook  # noqa: F401
        return
    except ImportError:
        pass
    if not os.path.exists(_AXON_SO):
        return
    try:
        lib = ctypes.CDLL(_AXON_SO)
    except OSError:
        return
    if not hasattr(lib, "axon_start_nrt_profile"):
        return
    lib.axon_start_nrt_profile.argtypes = [ctypes.POINTER(ctypes.c_int64), ctypes.c_size_t]
    lib.axon_start_nrt_profile.restype = ctypes.c_int64
    lib.axon_stop_nrt_profile.argtypes = [ctypes.c_char_p]
    lib.axon_stop_nrt_profile.restype = ctypes.c_int64

    @contextlib.contextmanager
    def _hook(output_dir, device_ids):
        import jax
        jax.devices()
        if device_ids:
            ids = (ctypes.c_int64 * len(device_ids))(*device_ids)
            rc = lib.axon_start_nrt_profile(ids, len(device_ids))
        else:
            rc = lib.axon_start_nrt_profile(None, 0)
        if rc != 0:
            raise RuntimeError(f"axon_start_nrt_profile rc={rc}")
        try:
            yield
        finally:
            n = lib.axon_stop_nrt_profile(str(output_dir).encode())
            if n < 0:
                raise RuntimeError(f"axon_stop_nrt_profile rc={n}")

    _the_hook = [_hook]
    mod = types.ModuleType("antenv.axon_hooks")
    mod.set_axon_ntff_profile_hook = lambda h: _the_hook.__setitem__(0, h)
    mod.get_axon_ntff_profile_hook = lambda: _the_hook[0]
    sys.modules["antenv.axon_hooks"] = mod
    import antenv
    antenv.axon_hooks = mod


def _split_excess_waits(nc, max_waits=1):
    """This walrus build allows one sync-wait per instruction; move extras
    onto same-engine NoOps inserted just before (execution order preserved)."""
    for f in nc.m.functions:
        for b in f.blocks:
            insts = b.instructions
            new = []
            changed = False
            for inst in insts:
                si = inst.sync_info
                ow = list(si.on_wait) if (si and si.on_wait) else []
                if len(ow) > max_waits:
                    extra, keep = ow[:-max_waits], ow[-max_waits:]
                    for k, w in enumerate(extra):
                        nop = mybir.InstNoOp(name=f"{inst.name}-w{k}", ins=[], outs=[])
                        nop.engine = inst.engine
                        nop.sync_info = mybir.SyncInfo(on_wait=[w], on_update=[])
                        new.append(nop)
                    inst.sync_info = mybir.SyncInfo(
                        on_wait=keep,
                        on_update=list(si.on_update) if si.on_update else [])
                    changed = True
                new.append(inst)
            if changed:
                b.instructions = new


def _build_nc(k_exp):
    f32 = mybir.dt.float32
    bf16 = mybir.dt.bfloat16
    fp8 = mybir.dt.float8e4
    Alu = mybir.AluOpType
    Act = mybir.ActivationFunctionType

    nc = bass.Bass(trn_type="TRN2", target_bir_lowering=False, debug=False)
    qT = nc.dram_tensor("qT", [128, (N // CH) * KT * CH], fp8, kind="ExternalInput")
    labd = nc.dram_tensor("lab", [N, 1], f32, kind="ExternalInput")
    antid = nc.dram_tensor("anti", [128, 128], f32, kind="ExternalInput")
    identd = nc.dram_tensor("ident", [128, 128], f32, kind="ExternalInput")
    outd = nc.dram_tensor("out", [128, 2], f32, kind="ExternalOutput")

    # exp(sim/T) where psum holds sim * 4**k_exp (quantization scale folded in)
    act_scale = 1.0 / (TEMP * float(4.0 ** k_exp))

    with tile.TileContext(nc) as tc, contextlib.ExitStack() as ctx:
        qp = ctx.enter_context(tc.tile_pool(name="qp", bufs=1))
        pp = ctx.enter_context(tc.tile_pool(name="pp", bufs=2, space="PSUM"))
        ep = ctx.enter_context(tc.tile_pool(name="ep", bufs=3))
        wp = ctx.enter_context(tc.tile_pool(name="wp", bufs=2))
        sp = ctx.enter_context(tc.tile_pool(name="sp", bufs=1))

        # ---- preload ----
        # small tensors first so the main-loop vector ops never wait behind
        # the 4.2MB qT stream
        lab_rows = sp.tile([128, NT, 1], f32)
        nc.sync.dma_start(
            out=lab_rows,
            in_=labd[PAD:PAD + R, :].rearrange("(t p) o -> p t o", p=128))
        # column labels broadcast to all partitions, cols [0, NT*128+WIN)
        labw_w = (NT - 1) * 128 + WIN        # 1536
        labw = sp.tile([128, labw_w], f32)
        nc.sync.dma_start(
            out=labw,
            in_=bass.AP(tensor=labd, offset=0, ap=[[0, 128], [1, labw_w]]))
        anti = sp.tile([128, 128], f32)
        nc.sync.dma_start(out=anti, in_=antid.ap())
        ident = sp.tile([128, 128], f32)
        nc.sync.dma_start(out=ident, in_=identd.ap())
        # qT chunks: [128, KT, CH] fp8, one per 512-col chunk
        qt = []
        for n in range(N // CH):
            q = qp.tile([128, KT, CH], fp8, tag=f"q{n}")
            nc.sync.dma_start(
                out=q, in_=qT[:, n * KT * CH:(n + 1) * KT * CH])
            qt.append(q)
        eps_t = sp.tile([128, 1], f32)
        nc.vector.memset(eps_t, EPS)
        warm = sp.tile([128, 128], bf16)
        nc.vector.memset(warm, 0.0)
        warm_ps = pp.tile([128, GRP], f32, tag="ps")
        for w in range(48):
            nc.tensor.matmul(warm_ps[:, :128], warm, warm, start=True, stop=True)

        # ---- accumulators ----
        dacc = sp.tile([128, NT * NG], f32)   # exp row-sums per (t, g)
        nacc = sp.tile([128, NT], f32)        # numerator per t
        edacc = sp.tile([128, NT], f32)       # diagonal exp per t

        # ---- main loop ----
        for t in range(NT):
            a = (PAD + t * 128) // CH        # lhsT chunk index
            off = (PAD + t * 128) % CH       # lhsT offset within chunk
            for g in range(NG):
                ps = pp.tile([128, GRP], f32, tag="ps")
                for sub in range(GRP // CH):
                    n = g * (GRP // CH) + sub
                    for k in range(0, KT, 2):
                        nc.tensor.matmul(
                            ps[:, sub * CH:(sub + 1) * CH],
                            qt[a][:, k:k + 2, off:off + 128],
                            qt[n][:, k:k + 2, :],
                            start=(k == 0), stop=(k == KT - 2),
                            perf_mode=mybir.MatmulPerfMode.DoubleRow)
                e = ep.tile([128, GRP], f32, tag="e")
                nc.scalar.activation(
                    out=e, in_=ps[:], func=Act.Exp, scale=act_scale,
                    accum_out=dacc[:, t * NG + g:t * NG + g + 1])
                if g == 0:
                    # window = cols [t*128, t*128+WIN) -- inside group 0
                    w0 = t * 128
                    u = wp.tile([128, WIN], f32, tag="u")
                    # u = (lab_col == lab_row) * exp(sim)
                    nc.vector.scalar_tensor_tensor(
                        out=u, in0=labw[:, w0:w0 + WIN],
                        scalar=lab_rows[:, t, :], in1=e[:, w0:w0 + WIN],
                        op0=Alu.is_equal, op1=Alu.mult)
                    # diagonal sits at window cols [PAD, PAD+128)
                    scr = wp.tile([128, 128], f32, tag="scr")
                    nc.vector.scalar_tensor_tensor(
                        out=scr, in0=u[:, PAD:PAD + 128], scalar=1.0,
                        in1=ident, op0=Alu.mult, op1=Alu.mult,
                        accum_out=edacc[:, t:t + 1])
                    nc.vector.tensor_tensor(
                        out=u[:, PAD:PAD + 128], in0=u[:, PAD:PAD + 128],
                        in1=anti, op=Alu.mult)
                    # numerator: sum over u where u > 1  (sim>0 gate)
                    scr2 = wp.tile([128, WIN], f32, tag="scr2")
                    nc.vector.scalar_tensor_tensor(
                        out=scr2, in0=u, scalar=1.0, in1=u,
                        op0=Alu.is_gt, op1=Alu.mult,
                        accum_out=nacc[:, t:t + 1])

        # ---- epilogue (all [128, NT]) ----
        dred = sp.tile([128, NT], f32)
        nc.vector.tensor_reduce(
            out=dred, in_=dacc.rearrange("p (t g) -> p t g", g=NG),
            axis=mybir.AxisListType.X, op=Alu.add)
        den = sp.tile([128, NT], f32)
        nc.vector.tensor_tensor(out=den, in0=dred, in1=edacc, op=Alu.subtract)
        v1 = sp.tile([128, NT], f32)
        nc.vector.tensor_scalar(out=v1, in0=nacc, scalar1=0.0, scalar2=None,
                                op0=Alu.is_gt)
        v2 = sp.tile([128, NT], f32)
        nc.vector.tensor_scalar(out=v2, in0=den, scalar1=0.0, scalar2=None,
                                op0=Alu.is_gt)
        v = sp.tile([128, NT], f32)
        nc.vector.tensor_tensor(out=v, in0=v1, in1=v2, op=Alu.mult)
        inv = sp.tile([128, NT], f32)
        nc.vector.tensor_scalar(out=inv, in0=v, scalar1=0.0, scalar2=None,
                                op0=Alu.is_equal)
        nsafe = sp.tile([128, NT], f32)
        nc.vector.tensor_tensor(out=nsafe, in0=nacc, in1=v, op=Alu.mult)
        nc.vector.tensor_tensor(out=nsafe, in0=nsafe, in1=inv, op=Alu.add)
        dsafe = sp.tile([128, NT], f32)
        nc.vector.tensor_tensor(out=dsafe, in0=den, in1=v, op=Alu.mult)
        nc.vector.tensor_tensor(out=dsafe, in0=dsafe, in1=inv, op=Alu.add)
        lgd = sp.tile([128, NT], f32)
        nc.scalar.activation(out=lgd, in_=dsafe, func=Act.Ln, bias=eps_t[:], scale=1.0)
        lgn = sp.tile([128, NT], f32)
        nc.scalar.activation(out=lgn, in_=nsafe, func=Act.Ln, scale=1.0)
        li = sp.tile([128, NT], f32)
        nc.vector.tensor_tensor(out=li, in0=lgd, in1=lgn, op=Alu.subtract)
        nc.vector.tensor_tensor(out=li, in0=li, in1=v, op=Alu.mult)
        o = sp.tile([128, 2], f32)
        nc.vector.tensor_reduce(out=o[:, 0:1], in_=li, axis=mybir.AxisListType.X,
                                op=Alu.add)
        nc.vector.tensor_reduce(out=o[:, 1:2], in_=v, axis=mybir.AxisListType.X,
                                op=Alu.add)
        nc.sync.dma_start(out=outd.ap(), in_=o)

    _split_excess_waits(nc)
    return nc


_NC_CACHE = {}


def _get_nc(k_exp):
    if k_exp not in _NC_CACHE:
        _NC_CACHE[k_exp] = _build_nc(k_exp)
    return _NC_CACHE[k_exp]


def _host_reference(emb, lab):
    """Numpy fallback (only for pathological label distributions where a
    class exceeds the PAD margin; never triggers for the target regime)."""
    e = emb / np.linalg.norm(emb, axis=1, keepdims=True).astype(np.float32)
    sim = (e @ e.T).astype(np.float32) / np.float32(TEMP)
    E = np.exp(sim, dtype=np.float32)
    pos = (lab[:, None] == lab[None, :]) & ~np.eye(len(lab), dtype=bool)
    valid = pos & (sim > 0)
    num = np.where(valid, E, 0).sum(1, dtype=np.float32)
    den = E.sum(1, dtype=np.float32) - np.diagonal(E)
    rv = valid.any(1) & (den > 0)
    ns = np.where(rv, num, np.float32(1.0))
    ds = np.where(rv, den, np.float32(1.0))
    li = np.log(ds + np.float32(EPS)) - np.log(ns)
    nv = int(rv.sum())
    if nv == 0:
        return np.float32(0.0)
    return np.float32(abs(float(np.where(rv, li, 0).sum(dtype=np.float64)) / nv))


def kernel(**inputs):
    global LAST_RESULTS
    emb = np.ascontiguousarray(np.asarray(inputs["embeddings"], dtype=np.float32))
    lab = np.asarray(inputs["labels"]).astype(np.int64).ravel()
    assert emb.shape == (N, D) and lab.shape == (N,)

    if np.bincount(lab, minlength=1).max() > PAD:
        return _host_reference(emb, lab)

    _install_axon_trace_hook()

    # host prep: normalize, sort by label, per-core roll + transpose
    e = emb / np.linalg.norm(emb, axis=1, keepdims=True).astype(np.float32)
    order = np.argsort(lab, kind="stable")
    es = np.ascontiguousarray(e[order])
    ls = lab[order].astype(np.float32)

    # fp8 e4m3 quantization with a power-of-two scale (keeps relative
    # precision exactly scale-invariant; act_scale compile-time per k_exp)
    absmax = float(np.abs(es).max())
    k_exp = int(np.floor(np.log2(240.0 / max(absmax, 1e-30))))
    k_exp = max(min(k_exp, 14), -14)
    es = (es * np.float32(2.0 ** k_exp)).astype(ml_dtypes.float8_e4m3)

    anti = (1.0 - np.eye(128, dtype=np.float32)).astype(np.float32)
    ident = np.eye(128, dtype=np.float32)

    in_maps = []
    for c in range(M):
        shift = c * R - PAD
        rolled = np.roll(es, -shift, axis=0)         # [N, D] fp8
        labr = np.roll(ls, -shift).reshape(N, 1)     # [N, 1] f32
        # [D, N] -> [128, NCH, KT, CH]: partition p, chunk n holds
        # qT[k*128+p, n*CH:(n+1)*CH] contiguous per (k)
        qTc = (rolled.T.reshape(KT, 128, N // CH, CH)
               .transpose(1, 2, 0, 3)
               .reshape(128, (N // CH) * KT * CH))
        qTc = np.ascontiguousarray(qTc)
        in_maps.append({
            "qT": qTc,
            "lab": np.ascontiguousarray(labr),
            "anti": anti,
            "ident": ident,
        })

    nc = _get_nc(k_exp)
    res = run_bass_kernel_spmd(nc, in_maps, core_ids=list(range(M)))
    LAST_RESULTS = res

    loss_sum = 0.0
    cnt = 0.0
    for c in range(M):
        o = res.results[c]["out"]
        loss_sum += float(o[:, 0].sum(dtype=np.float64))
        cnt += float(o[:, 1].sum(dtype=np.float64))
    if cnt <= 0:
        return np.float32(0.0)
    return np.float32(abs(loss_sum / cnt))



# revision 12
# speedup vs baseline: 1.8255x; 1.2891x over previous
"""Contrastive-loss Trainium2 kernel: circulant-band symmetric decomposition.

sim = e@e.T is symmetric, so each unordered pair is computed once: every
row computes only the 4096 columns circularly AHEAD of it (distance
1..4096 mod 8192; distance exactly 4096 only counted from the first-half
row). Rows are label-sorted and rolled per core as before, so each core's
[1024, 4224] band is contiguous in its local column space. Head (first
128) and tail (last 128) band blocks get strict-triangular edge masks.

Each pair (i,j) contributes exp to row i (free-axis sums: ACT accum_out /
DVE accum) AND to row j (partition-axis sums): per-tile bf16 exp bands are
shift-accumulated into a column accumulator on DVE, then one all-ones
matmul reduces partitions at the end. Positives (same label) only occur
within 256 ahead, so the numerator needs just the leading 384 band
columns. Final loss assembly (log, valid gating, mean) happens on host
from per-core row/col partials.

Matmuls run in fp8 e4m3 DoubleRow (power-of-two quantization scale folded
into the Exp activation scale); exp is stored bf16, row-sums kept fp32.
"""

import contextlib
import ctypes
import os
import sys
import types

import ml_dtypes
import numpy as np

import concourse.bass as bass
import concourse.mybir as mybir
import concourse.tile as tile
from concourse.bass_utils import run_bass_kernel_spmd

# problem constants (hardcoded per task contract)
N, D, NCLS = 8192, 512, 512
TEMP = 0.07
EPS = 1e-8
M = 8            # cores
R = N // M       # 1024 rows per core
NT = R // 128    # 8 row-tiles per core
PAD = 256        # roll margin; must exceed max class size
W = 4224         # band width: 128 head + 3968 middle + 128 tail
HALF = 4096      # circular half-distance
CSPAN = (NT - 1) * 128 + W   # 5120: col span touched by one core's bands
NSPAN = (NT - 1) * 128 + 384  # 1280: numerator col span
CH = 512         # qT chunk width
KT = D // 128    # 4 contraction tiles

_AXON_SO = "/opt/axon/libaxon_pjrt.so"

LAST_RESULTS = None   # BassKernelResults of the most recent run (for test.py)


def _install_axon_trace_hook():
    """Provide antenv.axon_hooks (NTFF profiling) if the image lacks it."""
    try:
        from antenv.axon_hooks import get_axon_ntff_profile_hook  # noqa: F401
        return
    except ImportError:
        pass
    if not os.path.exists(_AXON_SO):
        return
    try:
        lib = ctypes.CDLL(_AXON_SO)
    except OSError:
        return
    if not hasattr(lib, "axon_start_nrt_profile"):
        return
    lib.axon_start_nrt_profile.argtypes = [ctypes.POINTER(ctypes.c_int64), ctypes.c_size_t]
    lib.axon_start_nrt_profile.restype = ctypes.c_int64
    lib.axon_stop_nrt_profile.argtypes = [ctypes.c_char_p]
    lib.axon_stop_nrt_profile.restype = ctypes.c_int64

    @contextlib.contextmanager
    def _hook(output_dir, device_ids):
        import jax
        jax.devices()
        if device_ids:
            ids = (ctypes.c_int64 * len(device_ids))(*device_ids)
            rc = lib.axon_start_nrt_profile(ids, len(device_ids))
        else:
            rc = lib.axon_start_nrt_profile(None, 0)
        if rc != 0:
            raise RuntimeError(f"axon_start_nrt_profile rc={rc}")
        try:
            yield
        finally:
            n = lib.axon_stop_nrt_profile(str(output_dir).encode())
            if n < 0:
                raise RuntimeError(f"axon_stop_nrt_profile rc={n}")

    _the_hook = [_hook]
    mod = types.ModuleType("antenv.axon_hooks")
    mod.set_axon_ntff_profile_hook = lambda h: _the_hook.__setitem__(0, h)
    mod.get_axon_ntff_profile_hook = lambda: _the_hook[0]
    sys.modules["antenv.axon_hooks"] = mod
    import antenv
    antenv.axon_hooks = mod


def _split_excess_waits(nc, max_waits=1):
    """This walrus build allows one sync-wait per instruction; move extras
    onto same-engine NoOps inserted just before (execution order preserved)."""
    for f in nc.m.functions:
        for b in f.blocks:
            insts = b.instructions
            new = []
            changed = False
            for inst in insts:
                si = inst.sync_info
                ow = list(si.on_wait) if (si and si.on_wait) else []
                if len(ow) > max_waits:
                    extra, keep = ow[:-max_waits], ow[-max_waits:]
                    for k, w in enumerate(extra):
                        nop = mybir.InstNoOp(name=f"{inst.name}-w{k}", ins=[], outs=[])
                        nop.engine = inst.engine
                        nop.sync_info = mybir.SyncInfo(on_wait=[w], on_update=[])
                        new.append(nop)
                    inst.sync_info = mybir.SyncInfo(
                        on_wait=keep,
                        on_update=list(si.on_update) if si.on_update else [])
                    changed = True
                new.append(inst)
            if changed:
                b.instructions = new


def _build_nc(k_exp):
    f32 = mybir.dt.float32
    bf16 = mybir.dt.bfloat16
    fp8 = mybir.dt.float8e4
    Alu = mybir.AluOpType
    Act = mybir.ActivationFunctionType
    DR = mybir.MatmulPerfMode.DoubleRow

    nc = bass.Bass(trn_type="TRN2", target_bir_lowering=False, debug=False)
    qT = nc.dram_tensor("qT", [128, (N // CH) * KT * CH], fp8, kind="ExternalInput")
    labd = nc.dram_tensor("lab", [N, 1], f32, kind="ExternalInput")
    triud = nc.dram_tensor("triu", [128, 128], bf16, kind="ExternalInput")
    taild = nc.dram_tensor("tailm", [128, 128], bf16, kind="ExternalInput")
    outd = nc.dram_tensor("out", [128, 56], f32, kind="ExternalOutput")
    coldend = nc.dram_tensor("colden", [128, CSPAN], bf16, kind="ExternalOutput")
    colnumd = nc.dram_tensor("colnum", [128, NSPAN], bf16, kind="ExternalOutput")

    # exp(sim/T) where psum holds sim * 4**k_exp (quantization scale folded in)
    act_scale = 1.0 / (TEMP * float(4.0 ** k_exp))

    with tile.TileContext(nc) as tc, contextlib.ExitStack() as ctx:
        qp = ctx.enter_context(tc.tile_pool(name="qp", bufs=1))
        pp = ctx.enter_context(tc.tile_pool(name="pp", bufs=2, space="PSUM"))
        ph = ctx.enter_context(tc.tile_pool(name="ph", bufs=2, space="PSUM"))
        wp = ctx.enter_context(tc.tile_pool(name="wp", bufs=2))
        sp = ctx.enter_context(tc.tile_pool(name="sp", bufs=1))

        # ---- preload: small tensors first ----
        labw = sp.tile([128, NSPAN + 384], f32)   # local col labels [0, 1664)
        nc.sync.dma_start(
            out=labw,
            in_=bass.AP(tensor=labd, offset=0, ap=[[0, 128], [1, NSPAN + 384]]))
        lab_rows = sp.tile([128, NT, 1], f32)
        nc.sync.dma_start(
            out=lab_rows,
            in_=labd[PAD:PAD + R, :].rearrange("(t p) o -> p t o", p=128))
        triu = sp.tile([128, 128], bf16)
        nc.sync.dma_start(out=triu, in_=triud.ap())
        tailm = sp.tile([128, 128], bf16)
        nc.sync.dma_start(out=tailm, in_=taild.ap())
        qt = []
        for n in range(N // CH):
            q = qp.tile([128, KT, CH], fp8, tag=f"q{n}")
            nc.sync.dma_start(
                out=q, in_=qT[:, n * KT * CH:(n + 1) * KT * CH])
            qt.append(q)

        band = sp.tile([128, NT, W], bf16)
        colaccB = sp.tile([128, CSPAN], bf16)
        nc.vector.memset(colaccB, 0.0)
        colnumB = sp.tile([128, NSPAN], bf16)
        nc.vector.memset(colnumB, 0.0)
        acc = sp.tile([128, 56], f32)
        warm = sp.tile([128, 128], bf16)
        nc.vector.memset(warm, 0.0)
        warm_ps = pp.tile([128, 1024], f32, tag="ps")
        for w in range(48):
            nc.tensor.matmul(warm_ps[:, :128], warm, warm, start=True, stop=True)

        # ---- main loop: one 4224-wide band per 128-row tile ----
        for t in range(NT):
            base = PAD + t * 128          # abs local col of band start / row base
            a = base // CH                # lhsT chunk index
            off = base % CH               # lhsT offset within chunk
            lhs = qt[a]

            # head block [128,128] -> its own psum
            psh = ph.tile([128, 128], f32, tag="psh")
            for k in range(0, KT, 2):
                nc.tensor.matmul(
                    psh, lhs[:, k:k + 2, off:off + 128],
                    qt[base // CH][:, k:k + 2, base % CH:base % CH + 128],
                    start=(k == 0), stop=(k == KT - 2), perf_mode=DR)
            nc.scalar.activation(
                out=band[:, t, 0:128], in_=psh[:], func=Act.Exp, scale=act_scale)

            # 4 groups of 1024 cols: band offsets 128 + 1024*gi
            for gi in range(4):
                goff = 128 + 1024 * gi
                A = base + goff           # abs start
                ps = pp.tile([128, 1024], f32, tag="ps")
                s = A % CH
                subs = [(A, CH - s), (A + CH - s, CH)] if s else [(A, CH), (A + CH, CH)]
                if s:
                    subs.append((A + CH - s + CH, s))
                for (a0, wsub) in subs:
                    n0, co = a0 // CH, a0 % CH
                    pcol = a0 - A
                    for k in range(0, KT, 2):
                        nc.tensor.matmul(
                            ps[:, pcol:pcol + wsub],
                            lhs[:, k:k + 2, off:off + 128],
                            qt[n0][:, k:k + 2, co:co + wsub],
                            start=(k == 0), stop=(k == KT - 2), perf_mode=DR)
                if gi < 3:
                    nc.scalar.activation(
                        out=band[:, t, goff:goff + 1024], in_=ps[:], func=Act.Exp,
                        scale=act_scale, accum_out=acc[:, t * 4 + gi:t * 4 + gi + 1])
                else:
                    nc.scalar.activation(
                        out=band[:, t, goff:goff + 896], in_=ps[:, 0:896],
                        func=Act.Exp, scale=act_scale,
                        accum_out=acc[:, t * 4 + 3:t * 4 + 4])
                    nc.scalar.activation(
                        out=band[:, t, HALF:W], in_=ps[:, 896:1024],
                        func=Act.Exp, scale=act_scale)

            # edge masks in place; masked row-sums via accum
            nc.vector.scalar_tensor_tensor(
                out=band[:, t, 0:128], in0=band[:, t, 0:128], scalar=1.0,
                in1=triu, op0=Alu.mult, op1=Alu.mult,
                accum_out=acc[:, 32 + t:33 + t])
            nc.vector.scalar_tensor_tensor(
                out=band[:, t, HALF:W], in0=band[:, t, HALF:W], scalar=1.0,
                in1=tailm, op0=Alu.mult, op1=Alu.mult,
                accum_out=acc[:, 40 + t:41 + t])

            # numerator: same-label & exp>1 gate on leading 384 band cols
            u = wp.tile([128, 384], bf16, tag="u")
            nc.vector.scalar_tensor_tensor(
                out=u, in0=labw[:, base:base + 384], scalar=lab_rows[:, t, :],
                in1=band[:, t, 0:384], op0=Alu.is_equal, op1=Alu.mult)
            u2 = wp.tile([128, 384], bf16, tag="u2")
            nc.vector.scalar_tensor_tensor(
                out=u2, in0=u, scalar=1.0, in1=u, op0=Alu.is_gt, op1=Alu.mult,
                accum_out=acc[:, 48 + t:49 + t])

            # shift-accumulate column partials (bf16, DVE 2x)
            nc.vector.tensor_tensor(
                out=colaccB[:, t * 128:t * 128 + W],
                in0=colaccB[:, t * 128:t * 128 + W],
                in1=band[:, t, :], op=Alu.add)
            nc.vector.tensor_tensor(
                out=colnumB[:, t * 128:t * 128 + 384],
                in0=colnumB[:, t * 128:t * 128 + 384],
                in1=u2, op=Alu.add)

        # ---- ship column partials; host reduces the 128 partitions in f64 ----
        nc.sync.dma_start(out=coldend.ap(), in_=colaccB)
        nc.sync.dma_start(out=colnumd.ap(), in_=colnumB)
        nc.sync.dma_start(out=outd.ap(), in_=acc)

    _split_excess_waits(nc)
    return nc


_NC_CACHE = {}


def _get_nc(k_exp):
    if k_exp not in _NC_CACHE:
        _NC_CACHE[k_exp] = _build_nc(k_exp)
    return _NC_CACHE[k_exp]


def _host_reference(emb, lab):
    """Numpy fallback (only for pathological label distributions where a
    class exceeds the PAD margin; never triggers for the target regime)."""
    e = emb / np.linalg.norm(emb, axis=1, keepdims=True).astype(np.float32)
    sim = (e @ e.T).astype(np.float32) / np.float32(TEMP)
    E = np.exp(sim, dtype=np.float32)
    pos = (lab[:, None] == lab[None, :]) & ~np.eye(len(lab), dtype=bool)
    valid = pos & (sim > 0)
    num = np.where(valid, E, 0).sum(1, dtype=np.float32)
    den = E.sum(1, dtype=np.float32) - np.diagonal(E)
    rv = valid.any(1) & (den > 0)
    ns = np.where(rv, num, np.float32(1.0))
    ds = np.where(rv, den, np.float32(1.0))
    li = np.log(ds + np.float32(EPS)) - np.log(ns)
    nv = int(rv.sum())
    if nv == 0:
        return np.float32(0.0)
    return np.float32(abs(float(np.where(rv, li, 0).sum(dtype=np.float64)) / nv))


def kernel(**inputs):
    global LAST_RESULTS
    emb = np.ascontiguousarray(np.asarray(inputs["embeddings"], dtype=np.float32))
    lab = np.asarray(inputs["labels"]).astype(np.int64).ravel()
    assert emb.shape == (N, D) and lab.shape == (N,)

    if np.bincount(lab, minlength=1).max() > PAD:
        return _host_reference(emb, lab)

    _install_axon_trace_hook()

    # host prep: normalize, sort by label, per-core roll + transpose
    e = emb / np.linalg.norm(emb, axis=1, keepdims=True).astype(np.float32)
    order = np.argsort(lab, kind="stable")
    es = np.ascontiguousarray(e[order])
    ls = lab[order].astype(np.float32)

    # fp8 e4m3 quantization with a power-of-two scale (keeps relative
    # precision exactly scale-invariant; act_scale compile-time per k_exp)
    absmax = float(np.abs(es).max())
    k_exp = int(np.floor(np.log2(240.0 / max(absmax, 1e-30))))
    k_exp = max(min(k_exp, 14), -14)
    es = (es * np.float32(2.0 ** k_exp)).astype(ml_dtypes.float8_e4m3)

    ri = np.arange(128)
    triu = (ri[None, :] > ri[:, None]).astype(ml_dtypes.bfloat16)
    tail_incl = (ri[None, :] <= ri[:, None]).astype(ml_dtypes.bfloat16)
    tail_strict = (ri[None, :] < ri[:, None]).astype(ml_dtypes.bfloat16)

    in_maps = []
    for c in range(M):
        shift = c * R - PAD
        rolled = np.roll(es, -shift, axis=0)         # [N, D] fp8
        labr = np.roll(ls, -shift).reshape(N, 1)     # [N, 1] f32
        qTc = (rolled.T.reshape(KT, 128, N // CH, CH)
               .transpose(1, 2, 0, 3)
               .reshape(128, (N // CH) * KT * CH))
        qTc = np.ascontiguousarray(qTc)
        in_maps.append({
            "qT": qTc,
            "lab": np.ascontiguousarray(labr),
            "triu": triu,
            "tailm": tail_incl if c < 4 else tail_strict,
        })

    nc = _get_nc(k_exp)
    res = run_bass_kernel_spmd(nc, in_maps, core_ids=list(range(M)))
    LAST_RESULTS = res

    # ---- host assembly of row/col partials ----
    num = np.zeros(N, np.float64)
    den = np.zeros(N, np.float64)
    idx = np.arange(CSPAN)
    for c in range(M):
        o = res.results[c]["out"].astype(np.float64)       # [128, 56]
        den_rows = (o[:, :32].reshape(128, NT, 4).sum(2)
                    + o[:, 32:40] + o[:, 40:48])           # [p, t]
        num_rows = o[:, 48:56]
        g0 = c * R
        den[g0:g0 + R] += den_rows.T.ravel()
        num[g0:g0 + R] += num_rows.T.ravel()
        jj = (g0 + idx) % N
        np.add.at(den, jj,
                  res.results[c]["colden"].astype(np.float64).sum(0))
        np.add.at(num, jj[:NSPAN],
                  res.results[c]["colnum"].astype(np.float64).sum(0))

    valid = (num > 0) & (den > 0)
    nv = int(valid.sum())
    if nv == 0:
        return np.float32(0.0)
    ns = np.where(valid, num, 1.0)
    ds = np.where(valid, den, 1.0)
    li = np.log(ds + EPS) - np.log(ns)
    # undo the label sort: row partials are in sorted order
    return np.float32(abs(float(li[valid].sum()) / nv))


# revision 14
# speedup vs baseline: 1.9934x; 1.0920x over previous
"""Contrastive-loss Trainium2 kernel: circulant-band symmetric decomposition.

sim = e@e.T is symmetric, so each unordered pair is computed once: every
row computes only the 4096 columns circularly AHEAD of it (distance
1..4096 mod 8192; distance exactly 4096 only counted from the first-half
row). Rows are label-sorted and rolled per core as before, so each core's
[1024, 4224] band is contiguous in its local column space. Head (first
128) and tail (last 128) band blocks get strict-triangular edge masks.

Each pair (i,j) contributes exp to row i (free-axis sums: ACT accum_out /
DVE accum) AND to row j (partition-axis sums): per-tile bf16 exp bands are
shift-accumulated into a column accumulator on DVE, then one all-ones
matmul reduces partitions at the end. Positives (same label) only occur
within 256 ahead, so the numerator needs just the leading 384 band
columns. Final loss assembly (log, valid gating, mean) happens on host
from per-core row/col partials.

Matmuls run in fp8 e4m3 DoubleRow (power-of-two quantization scale folded
into the Exp activation scale); exp is stored bf16, row-sums kept fp32.
"""

import contextlib
import ctypes
import os
import sys
import types

import ml_dtypes
import numpy as np

import concourse.bass as bass
import concourse.mybir as mybir
import concourse.tile as tile
from concourse.bass_utils import run_bass_kernel_spmd

# problem constants (hardcoded per task contract)
N, D, NCLS = 8192, 512, 512
TEMP = 0.07
EPS = 1e-8
M = 8            # cores
R = N // M       # 1024 rows per core
NT = R // 128    # 8 row-tiles per core
PAD = 256        # roll margin; must exceed max class size
W = 4224         # band width: 128 head + 3968 middle + 128 tail
HALF = 4096      # circular half-distance
CSPAN = (NT - 1) * 128 + W   # 5120: col span touched by one core's bands
NSPAN = (NT - 1) * 128 + 384  # 1280: numerator col span
CH = 512         # qT chunk width
KT = D // 128    # 4 contraction tiles

_AXON_SO = "/opt/axon/libaxon_pjrt.so"

LAST_RESULTS = None   # BassKernelResults of the most recent run (for test.py)


def _install_axon_trace_hook():
    """Provide antenv.axon_hooks (NTFF profiling) if the image lacks it."""
    try:
        from antenv.axon_hooks import get_axon_ntff_profile_hook  # noqa: F401
        return
    except ImportError:
        pass
    if not os.path.exists(_AXON_SO):
        return
    try:
        lib = ctypes.CDLL(_AXON_SO)
    except OSError:
        return
    if not hasattr(lib, "axon_start_nrt_profile"):
        return
    lib.axon_start_nrt_profile.argtypes = [ctypes.POINTER(ctypes.c_int64), ctypes.c_size_t]
    lib.axon_start_nrt_profile.restype = ctypes.c_int64
    lib.axon_stop_nrt_profile.argtypes = [ctypes.c_char_p]
    lib.axon_stop_nrt_profile.restype = ctypes.c_int64

    @contextlib.contextmanager
    def _hook(output_dir, device_ids):
        import jax
        jax.devices()
        if device_ids:
            ids = (ctypes.c_int64 * len(device_ids))(*device_ids)
            rc = lib.axon_start_nrt_profile(ids, len(device_ids))
        else:
            rc = lib.axon_start_nrt_profile(None, 0)
        if rc != 0:
            raise RuntimeError(f"axon_start_nrt_profile rc={rc}")
        try:
            yield
        finally:
            n = lib.axon_stop_nrt_profile(str(output_dir).encode())
            if n < 0:
                raise RuntimeError(f"axon_stop_nrt_profile rc={n}")

    _the_hook = [_hook]
    mod = types.ModuleType("antenv.axon_hooks")
    mod.set_axon_ntff_profile_hook = lambda h: _the_hook.__setitem__(0, h)
    mod.get_axon_ntff_profile_hook = lambda: _the_hook[0]
    sys.modules["antenv.axon_hooks"] = mod
    import antenv
    antenv.axon_hooks = mod


def _split_excess_waits(nc, max_waits=1):
    """This walrus build allows one sync-wait per instruction; move extras
    onto same-engine NoOps inserted just before (execution order preserved)."""
    for f in nc.m.functions:
        for b in f.blocks:
            insts = b.instructions
            new = []
            changed = False
            for inst in insts:
                si = inst.sync_info
                ow = list(si.on_wait) if (si and si.on_wait) else []
                if len(ow) > max_waits:
                    extra, keep = ow[:-max_waits], ow[-max_waits:]
                    for k, w in enumerate(extra):
                        nop = mybir.InstNoOp(name=f"{inst.name}-w{k}", ins=[], outs=[])
                        nop.engine = inst.engine
                        nop.sync_info = mybir.SyncInfo(on_wait=[w], on_update=[])
                        new.append(nop)
                    inst.sync_info = mybir.SyncInfo(
                        on_wait=keep,
                        on_update=list(si.on_update) if si.on_update else [])
                    changed = True
                new.append(inst)
            if changed:
                b.instructions = new


def _build_nc(k_exp):
    f32 = mybir.dt.float32
    bf16 = mybir.dt.bfloat16
    fp8 = mybir.dt.float8e4
    Alu = mybir.AluOpType
    Act = mybir.ActivationFunctionType
    DR = mybir.MatmulPerfMode.DoubleRow

    nc = bass.Bass(trn_type="TRN2", target_bir_lowering=False, debug=False)
    qT = nc.dram_tensor("qT", [128, (N // CH) * KT * CH], fp8, kind="ExternalInput")
    labd = nc.dram_tensor("lab", [N, 1], f32, kind="ExternalInput")
    triud = nc.dram_tensor("triu", [128, 128], bf16, kind="ExternalInput")
    taild = nc.dram_tensor("tailm", [128, 128], bf16, kind="ExternalInput")
    outd = nc.dram_tensor("out", [128, 56], f32, kind="ExternalOutput")
    coldend = nc.dram_tensor("colden", [128, CSPAN], bf16, kind="ExternalOutput")
    colnumd = nc.dram_tensor("colnum", [128, NSPAN], bf16, kind="ExternalOutput")

    # exp(sim/T) where psum holds sim * 4**k_exp (quantization scale folded in)
    act_scale = 1.0 / (TEMP * float(4.0 ** k_exp))

    with tile.TileContext(nc) as tc, contextlib.ExitStack() as ctx:
        qp = ctx.enter_context(tc.tile_pool(name="qp", bufs=1))
        pp = ctx.enter_context(tc.tile_pool(name="pp", bufs=3, space="PSUM"))
        ph = ctx.enter_context(tc.tile_pool(name="ph", bufs=2, space="PSUM"))
        wp = ctx.enter_context(tc.tile_pool(name="wp", bufs=2))
        sp = ctx.enter_context(tc.tile_pool(name="sp", bufs=1))

        # ---- preload: small tensors first ----
        labw = sp.tile([128, NSPAN + 384], f32)   # local col labels [0, 1664)
        nc.sync.dma_start(
            out=labw,
            in_=bass.AP(tensor=labd, offset=0, ap=[[0, 128], [1, NSPAN + 384]]))
        lab_rows = sp.tile([128, NT, 1], f32)
        nc.sync.dma_start(
            out=lab_rows,
            in_=labd[PAD:PAD + R, :].rearrange("(t p) o -> p t o", p=128))
        triu = sp.tile([128, 128], bf16)
        nc.sync.dma_start(out=triu, in_=triud.ap())
        tailm = sp.tile([128, 128], bf16)
        nc.sync.dma_start(out=tailm, in_=taild.ap())
        qt = []
        for n in range(N // CH):
            q = qp.tile([128, KT, CH], fp8, tag=f"q{n}")
            nc.sync.dma_start(
                out=q, in_=qT[:, n * KT * CH:(n + 1) * KT * CH])
            qt.append(q)

        band = sp.tile([128, NT, W], bf16)
        colaccB = sp.tile([128, CSPAN], bf16)
        nc.vector.memset(colaccB, 0.0)
        colnumB = sp.tile([128, NSPAN], bf16)
        nc.vector.memset(colnumB, 0.0)
        acc = sp.tile([128, 56], f32)
        warm = sp.tile([128, 128], bf16)
        nc.vector.memset(warm, 0.0)
        warm_ps = pp.tile([128, 1024], f32, tag="ps")
        for w in range(48):
            nc.tensor.matmul(warm_ps[:, :128], warm, warm, start=True, stop=True)

        # ---- main loop: one 4224-wide band per 128-row tile ----
        for t in range(NT):
            base = PAD + t * 128          # abs local col of band start / row base
            a = base // CH                # lhsT chunk index
            off = base % CH               # lhsT offset within chunk
            lhs = qt[a]

            # head block [128,128] -> its own psum
            psh = ph.tile([128, 128], f32, tag="psh")
            for k in range(0, KT, 2):
                nc.tensor.matmul(
                    psh, lhs[:, k:k + 2, off:off + 128],
                    qt[base // CH][:, k:k + 2, base % CH:base % CH + 128],
                    start=(k == 0), stop=(k == KT - 2), perf_mode=DR)
            nc.scalar.activation(
                out=band[:, t, 0:128], in_=psh[:], func=Act.Exp, scale=act_scale)

            # 4 groups of 1024 cols: band offsets 128 + 1024*gi
            for gi in range(4):
                goff = 128 + 1024 * gi
                A = base + goff           # abs start
                ps = pp.tile([128, 1024], f32, tag="ps")
                s = A % CH
                subs = [(A, CH - s), (A + CH - s, CH)] if s else [(A, CH), (A + CH, CH)]
                if s:
                    subs.append((A + CH - s + CH, s))
                for (a0, wsub) in subs:
                    n0, co = a0 // CH, a0 % CH
                    pcol = a0 - A
                    for k in range(0, KT, 2):
                        nc.tensor.matmul(
                            ps[:, pcol:pcol + wsub],
                            lhs[:, k:k + 2, off:off + 128],
                            qt[n0][:, k:k + 2, co:co + wsub],
                            start=(k == 0), stop=(k == KT - 2), perf_mode=DR)
                if gi < 3:
                    nc.scalar.activation(
                        out=band[:, t, goff:goff + 1024], in_=ps[:], func=Act.Exp,
                        scale=act_scale, accum_out=acc[:, t * 4 + gi:t * 4 + gi + 1])
                else:
                    nc.scalar.activation(
                        out=band[:, t, goff:goff + 896], in_=ps[:, 0:896],
                        func=Act.Exp, scale=act_scale,
                        accum_out=acc[:, t * 4 + 3:t * 4 + 4])
                    nc.scalar.activation(
                        out=band[:, t, HALF:W], in_=ps[:, 896:1024],
                        func=Act.Exp, scale=act_scale)

            # edge masks in place; masked row-sums via accum
            nc.vector.scalar_tensor_tensor(
                out=band[:, t, 0:128], in0=band[:, t, 0:128], scalar=1.0,
                in1=triu, op0=Alu.mult, op1=Alu.mult,
                accum_out=acc[:, 32 + t:33 + t])
            nc.vector.scalar_tensor_tensor(
                out=band[:, t, HALF:W], in0=band[:, t, HALF:W], scalar=1.0,
                in1=tailm, op0=Alu.mult, op1=Alu.mult,
                accum_out=acc[:, 40 + t:41 + t])

            # numerator: same-label & exp>1 gate on leading 384 band cols
            u = wp.tile([128, 384], bf16, tag="u")
            nc.vector.scalar_tensor_tensor(
                out=u, in0=labw[:, base:base + 384], scalar=lab_rows[:, t, :],
                in1=band[:, t, 0:384], op0=Alu.is_equal, op1=Alu.mult)
            u2 = wp.tile([128, 384], bf16, tag="u2")
            nc.vector.scalar_tensor_tensor(
                out=u2, in0=u, scalar=1.0, in1=u, op0=Alu.is_gt, op1=Alu.mult,
                accum_out=acc[:, 48 + t:49 + t])

            # shift-accumulate column partials (bf16, DVE 2x)
            nc.vector.tensor_tensor(
                out=colaccB[:, t * 128:t * 128 + W],
                in0=colaccB[:, t * 128:t * 128 + W],
                in1=band[:, t, :], op=Alu.add)
            nc.vector.tensor_tensor(
                out=colnumB[:, t * 128:t * 128 + 384],
                in0=colnumB[:, t * 128:t * 128 + 384],
                in1=u2, op=Alu.add)

        # ---- ship column partials; host reduces the 128 partitions in f64 ----
        nc.sync.dma_start(out=coldend.ap(), in_=colaccB)
        nc.sync.dma_start(out=colnumd.ap(), in_=colnumB)
        nc.sync.dma_start(out=outd.ap(), in_=acc)

    _split_excess_waits(nc)
    return nc


_NC_CACHE = {}


def _get_nc(k_exp):
    if k_exp not in _NC_CACHE:
        _NC_CACHE[k_exp] = _build_nc(k_exp)
    return _NC_CACHE[k_exp]


def _host_reference(emb, lab):
    """Numpy fallback (only for pathological label distributions where a
    class exceeds the PAD margin; never triggers for the target regime)."""
    e = emb / np.linalg.norm(emb, axis=1, keepdims=True).astype(np.float32)
    sim = (e @ e.T).astype(np.float32) / np.float32(TEMP)
    E = np.exp(sim, dtype=np.float32)
    pos = (lab[:, None] == lab[None, :]) & ~np.eye(len(lab), dtype=bool)
    valid = pos & (sim > 0)
    num = np.where(valid, E, 0).sum(1, dtype=np.float32)
    den = E.sum(1, dtype=np.float32) - np.diagonal(E)
    rv = valid.any(1) & (den > 0)
    ns = np.where(rv, num, np.float32(1.0))
    ds = np.where(rv, den, np.float32(1.0))
    li = np.log(ds + np.float32(EPS)) - np.log(ns)
    nv = int(rv.sum())
    if nv == 0:
        return np.float32(0.0)
    return np.float32(abs(float(np.where(rv, li, 0).sum(dtype=np.float64)) / nv))


def kernel(**inputs):
    global LAST_RESULTS
    emb = np.ascontiguousarray(np.asarray(inputs["embeddings"], dtype=np.float32))
    lab = np.asarray(inputs["labels"]).astype(np.int64).ravel()
    assert emb.shape == (N, D) and lab.shape == (N,)

    if np.bincount(lab, minlength=1).max() > PAD:
        return _host_reference(emb, lab)

    _install_axon_trace_hook()

    # host prep: normalize, sort by label, per-core roll + transpose
    e = emb / np.linalg.norm(emb, axis=1, keepdims=True).astype(np.float32)
    order = np.argsort(lab, kind="stable")
    es = np.ascontiguousarray(e[order])
    ls = lab[order].astype(np.float32)

    # fp8 e4m3 quantization with a power-of-two scale (keeps relative
    # precision exactly scale-invariant; act_scale compile-time per k_exp)
    absmax = float(np.abs(es).max())
    k_exp = int(np.floor(np.log2(240.0 / max(absmax, 1e-30))))
    k_exp = max(min(k_exp, 14), -14)
    es = (es * np.float32(2.0 ** k_exp)).astype(ml_dtypes.float8_e4m3)

    ri = np.arange(128)
    triu = (ri[None, :] > ri[:, None]).astype(ml_dtypes.bfloat16)
    tail_incl = (ri[None, :] <= ri[:, None]).astype(ml_dtypes.bfloat16)
    tail_strict = (ri[None, :] < ri[:, None]).astype(ml_dtypes.bfloat16)

    in_maps = []
    for c in range(M):
        shift = c * R - PAD
        rolled = np.roll(es, -shift, axis=0)         # [N, D] fp8
        labr = np.roll(ls, -shift).reshape(N, 1)     # [N, 1] f32
        qTc = (rolled.T.reshape(KT, 128, N // CH, CH)
               .transpose(1, 2, 0, 3)
               .reshape(128, (N // CH) * KT * CH))
        qTc = np.ascontiguousarray(qTc)
        in_maps.append({
            "qT": qTc,
            "lab": np.ascontiguousarray(labr),
            "triu": triu,
            "tailm": tail_incl if c < 4 else tail_strict,
        })

    nc = _get_nc(k_exp)
    res = run_bass_kernel_spmd(nc, in_maps, core_ids=list(range(M)))
    LAST_RESULTS = res

    # ---- host assembly of row/col partials ----
    num = np.zeros(N, np.float64)
    den = np.zeros(N, np.float64)
    idx = np.arange(CSPAN)
    for c in range(M):
        o = res.results[c]["out"].astype(np.float64)       # [128, 56]
        den_rows = (o[:, :32].reshape(128, NT, 4).sum(2)
                    + o[:, 32:40] + o[:, 40:48])           # [p, t]
        num_rows = o[:, 48:56]
        g0 = c * R
        den[g0:g0 + R] += den_rows.T.ravel()
        num[g0:g0 + R] += num_rows.T.ravel()
        jj = (g0 + idx) % N
        np.add.at(den, jj,
                  res.results[c]["colden"].astype(np.float64).sum(0))
        np.add.at(num, jj[:NSPAN],
                  res.results[c]["colnum"].astype(np.float64).sum(0))

    global LAST_PARTIALS
    LAST_PARTIALS = (num.copy(), den.copy())
    valid = (num > 0) & (den > 0)
    nv = int(valid.sum())
    if nv == 0:
        return np.float32(0.0)
    ns = np.where(valid, num, 1.0)
    ds = np.where(valid, den, 1.0)
    li = np.log(ds + EPS) - np.log(ns)
    return np.float32(abs(float(li[valid].sum()) / nv))
